# revision 35
# baseline (speedup 1.0000x reference)
"""Trainium2 Bass kernel for nn_BulkSpaceGenerator.

Math: the fast-marching scan g_k = g_{k-1} + (1/(k+1))(c_k - g_{k-1}) starting
from c_0 yields the running mean g_k = mean(c_0..c_k); the mean over k of those
is sum_j w_j c_j with w_j = (1/K)(H_K - H_j) (harmonic numbers). Since
c_j = tokens @ W[:, j*D:(j+1)*D] + b[j*D:(j+1)*D], the whole module is

    out = tokens @ W_eff + b_eff,   W_eff = sum_j w_j W_j,  b_eff = sum_j w_j b_j

W_eff/b_eff are constant-folded from the weights on the host during input
formatting (the same pass that casts to f16 and transposes); the device then
runs the (8192x1024)@(1024x1024) matmul on the PE array at the f16 roofline.

Sharding: pure data-parallel over tokens (8 shards of 1024 tokens); W_eff
(2.1MB f16) is replicated. Per-core stream is only ~4.2MB in + 2.1MB out, so
the DMA feed finishes long before the PE does and the kernel is PE-bound
(~216ns per 128x512 matmul, 128 matmuls).

Schedule:
  - W_eff halves lead the sync ring; token k-tiles alternate across both
    HWDGE rings; the (tiny) folded bias rides late on the scalar ring.
  - psum generations, kt-outer: m-half 0 as one 8-bank generation (one bank
    per d-tile), m-half 1 as two 4-bank sub-generations so eviction work
    pipelines into the stream and only four evictions trail it.
  - evictions (psum + bias -> f16): even d-tiles on ACT -> scalar-ring DMA,
    odd d-tiles on DVE -> sync-ring DMA.

Layout per core c (tokens rows c*1024:(c+1)*1024):
  tokT : (1024, 1024) f16 -- tokens^T slice (k, m)
  weff : (128, 8192)  f16 -- W_eff[kt*128+p, d] at [p, kt*1024+d], replicated
  beff : (128, 8)     f32 -- b_eff[dt*128+p] at [p, dt]
  outT : (1024, 1024) f16 -- out^T slice (d, m); host reassembles
"""

import os
from contextlib import ExitStack

import numpy as np

import concourse.bass as bass
import concourse.tile as tile
from concourse import bacc, mybir
from concourse.bass_utils import run_bass_kernel_spmd

D_MODEL = 1024
BULK_DIM = 10
B, N = 4, 2048
BN = B * N                     # 8192 tokens
NCORES = 8
MS = BN // NCORES              # 1024 tokens per core
KT = D_MODEL // 128            # 8 contraction k-tiles
NDT = D_MODEL // 128           # 8 output d-tiles per core
MCHUNK = 512                   # moving free dim per matmul

# w_j = (1/K) * (H_K - H_j), H_j = sum_{i=1..j} 1/i
_H = np.cumsum(1.0 / np.arange(1, BULK_DIM + 1))
W_COEF = ((_H[-1] - np.concatenate([[0.0], _H[:-1]])) / BULK_DIM).tolist()

MODE = os.environ.get("BULK_KERNEL_MODE", "host")

_BUILD_CACHE = {}

N_PREWARM = 10                 # PE warm-up no-op matmuls before the stream


def _build(mode: str) -> bass.Bass:
    f32 = mybir.dt.float32
    f16 = mybir.dt.float16

    nc = bacc.Bacc("TRN2", target_bir_lowering=False, debug=False,
                   num_devices=NCORES)
    tokT = nc.dram_tensor("tokT", [D_MODEL, MS], f16,
                          kind="ExternalInput").ap()
    weff = nc.dram_tensor("weff", [128, KT * D_MODEL], f16,
                          kind="ExternalInput").ap()
    beff = nc.dram_tensor("beff", [128, NDT], f32, kind="ExternalInput").ap()
    outT = nc.dram_tensor("outT", [D_MODEL, MS], f16,
                          kind="ExternalOutput").ap()

    with tile.TileContext(nc) as tc, ExitStack() as ctx:
        weff_pool = ctx.enter_context(tc.tile_pool(name="weff", bufs=1))
        tok_pool = ctx.enter_context(tc.tile_pool(name="tok", bufs=KT))
        bias_pool = ctx.enter_context(tc.tile_pool(name="bias", bufs=1))
        zero_pool = ctx.enter_context(tc.tile_pool(name="zero", bufs=2))
        psum_pool = ctx.enter_context(
            tc.tile_pool(name="psum", bufs=8, space="PSUM"))
        out_pool = ctx.enter_context(tc.tile_pool(name="osb", bufs=16))

        # W_eff halves lead the sync ring; tokens alternate across both rings
        wt = weff_pool.tile([128, KT * D_MODEL], f16, tag="wt")
        half = KT * D_MODEL // 2
        nc.sync.dma_start(wt[:, 0:half], weff[:, 0:half])
        nc.sync.dma_start(wt[:, half:], weff[:, half:])

        toks = []
        for kt in range(KT):
            tk = tok_pool.tile([128, MS], f16, tag="tk")
            eng = nc.scalar if kt % 2 == 0 else nc.sync
            eng.dma_start(tk[:], tokT[kt * 128:(kt + 1) * 128, :])
            toks.append(tk)

        bt = bias_pool.tile([128, NDT], f32, tag="bt")
        nc.scalar.dma_start(bt[:], beff[:, :])

        zmm = zero_pool.tile([128, 128], f16, tag="zmm")
        nc.vector.memset(zmm[:], 0.0)
        zrhs = zero_pool.tile([128, MCHUNK], f16, tag="zrhs")
        nc.vector.memset(zrhs[:], 0.0)

        def evict(ps, dt_i, mi):
            ot = out_pool.tile([128, MCHUNK], f16, name="ot", tag="ot")
            dsl = slice(dt_i * 128, (dt_i + 1) * 128)
            msl = slice(mi * MCHUNK, (mi + 1) * MCHUNK)
            if dt_i % 2 == 0:
                nc.scalar.add(ot[:], ps[:], bt[:, dt_i:dt_i + 1])
                nc.scalar.dma_start(outT[dsl, msl], ot[:])
            else:
                nc.vector.tensor_scalar_add(ot[:], ps[:],
                                            bt[:, dt_i:dt_i + 1])
                nc.sync.dma_start(outT[dsl, msl], ot[:])

        def gen(mi, dts, prewarm):
            psums = [psum_pool.tile([128, MCHUNK], f32, name="ps", tag="ps")
                     for _ in dts]
            if prewarm:
                for _ in range(N_PREWARM):
                    nc.tensor.matmul(psums[0][:], lhsT=zmm[:], rhs=zrhs[:],
                                     start=False, stop=False)
            for kt in range(KT):
                for i, dt_i in enumerate(dts):
                    lhsT = wt[:, kt * D_MODEL + dt_i * 128:
                              kt * D_MODEL + (dt_i + 1) * 128]
                    nc.tensor.matmul(
                        psums[i][:], lhsT=lhsT,
                        rhs=toks[kt][:, mi * MCHUNK:(mi + 1) * MCHUNK],
                        start=(kt == 0), stop=(kt == KT - 1))
            for i, dt_i in enumerate(dts):
                evict(psums[i], dt_i, mi)

        # m-half 0: all 8 d-tiles at once; m-half 1: two 4-bank sub-gens
        gen(0, list(range(NDT)), prewarm=True)
        gen(1, [0, 1, 2, 3], prewarm=False)
        gen(1, [4, 5, 6, 7], prewarm=False)

    nc.compile()
    return nc


def _get_nc(mode: str) -> bass.Bass:
    if mode not in _BUILD_CACHE:
        _BUILD_CACHE[mode] = _build(mode)
    return _BUILD_CACHE[mode]


def _make_in_maps(boundary_tokens, W_b2b, b_b2b):
    wcoef = np.asarray(W_COEF, dtype=np.float32)
    tok = np.asarray(boundary_tokens, dtype=np.float32).reshape(BN, D_MODEL)
    # constant-fold the scan into the weights: W_eff = sum_j w_j W_j
    Weff = (np.asarray(W_b2b, dtype=np.float32).reshape(
        D_MODEL, BULK_DIM, D_MODEL) * wcoef[None, :, None]).sum(
        axis=1, dtype=np.float32)
    beff = (np.asarray(b_b2b, dtype=np.float32).reshape(BULK_DIM, D_MODEL)
            * wcoef[:, None]).sum(axis=0, dtype=np.float32)
    wmap = np.ascontiguousarray(
        Weff.astype(np.float16).reshape(KT, 128, D_MODEL)
        .transpose(1, 0, 2).reshape(128, KT * D_MODEL))
    bmap = np.ascontiguousarray(beff.reshape(NDT, 128).T.astype(np.float32))
    in_maps = []
    for c in range(NCORES):
        in_maps.append({
            "tokT": np.ascontiguousarray(
                tok[c * MS:(c + 1) * MS, :].T.astype(np.float16)),
            "weff": wmap,
            "beff": bmap,
        })
    return in_maps


def _assemble(results):
    out = np.empty((BN, D_MODEL), dtype=np.float32)
    for c in range(NCORES):
        out[c * MS:(c + 1) * MS, :] = results[c]["outT"].T.astype(np.float32)
    return out.reshape(B, N, D_MODEL)


def run(boundary_tokens, W_b2b, b_b2b, mode=None, **spmd_kwargs):
    mode = mode or MODE
    nc = _get_nc(mode)
    in_maps = _make_in_maps(boundary_tokens, W_b2b, b_b2b)
    res = run_bass_kernel_spmd(nc, in_maps, list(range(NCORES)), **spmd_kwargs)
    return _assemble(res.results), res


def kernel(boundary_tokens, W_b2b, b_b2b):
    out, _ = run(boundary_tokens, W_b2b, b_b2b)
    return out


# revision 40
# speedup vs baseline: 1.0405x; 1.0405x over previous
"""Trainium2 Bass kernel for nn_BulkSpaceGenerator.

Math: the fast-marching scan g_k = g_{k-1} + (1/(k+1))(c_k - g_{k-1}) starting
from c_0 yields the running mean g_k = mean(c_0..c_k); the mean over k of those
is sum_j w_j c_j with w_j = (1/K)(H_K - H_j) (harmonic numbers). Since
c_j = tokens @ W[:, j*D:(j+1)*D] + b[j*D:(j+1)*D], the whole module is

    out = tokens @ W_eff + b_eff,   W_eff = sum_j w_j W_j,  b_eff = sum_j w_j b_j

W_eff/b_eff are constant-folded from the weights on the host during input
formatting (the same pass that casts to f16 and transposes); the device then
runs the (8192x1024)@(1024x1024) matmul on the PE array at the f16 roofline.

Sharding: pure data-parallel over tokens (8 shards of 1024 tokens); W_eff
(2.1MB f16) is replicated. Per-core stream is only ~4.2MB in + 2.1MB out, so
the DMA feed finishes long before the PE does and the kernel is PE-bound
(~216ns per 128x512 matmul, 128 matmuls).

Schedule:
  - W_eff halves lead the sync ring; token k-tiles alternate across both
    HWDGE rings; the (tiny) folded bias rides late on the scalar ring.
  - psum generations, kt-outer: m-half 0 as one 8-bank generation (one bank
    per d-tile), m-half 1 as two 4-bank sub-generations so eviction work
    pipelines into the stream and only four evictions trail it.
  - evictions (psum + bias -> f16): even d-tiles on ACT -> scalar-ring DMA,
    odd d-tiles on DVE -> sync-ring DMA.

Layout per core c (tokens rows c*1024:(c+1)*1024):
  tokT : (1024, 1024) f16 -- tokens^T slice (k, m)
  weff : (128, 8192)  f16 -- W_eff[kt*128+p, d] at [p, kt*1024+d], replicated
  beff : (128, 8)     f32 -- b_eff[dt*128+p] at [p, dt]
  outT : (1024, 1024) f16 -- out^T slice (d, m); host reassembles
"""

import os
from contextlib import ExitStack

import numpy as np

import concourse.bass as bass
import concourse.tile as tile
from concourse import bacc, mybir
from concourse.bass_utils import run_bass_kernel_spmd

D_MODEL = 1024
BULK_DIM = 10
B, N = 4, 2048
BN = B * N                     # 8192 tokens
NCORES = 8
MS = BN // NCORES              # 1024 tokens per core
KT = D_MODEL // 128            # 8 contraction k-tiles
NDT = D_MODEL // 128           # 8 output d-tiles per core
MCHUNK = 512                   # moving free dim per matmul

# w_j = (1/K) * (H_K - H_j), H_j = sum_{i=1..j} 1/i
_H = np.cumsum(1.0 / np.arange(1, BULK_DIM + 1))
W_COEF = ((_H[-1] - np.concatenate([[0.0], _H[:-1]])) / BULK_DIM).tolist()

MODE = os.environ.get("BULK_KERNEL_MODE", "host")

_BUILD_CACHE = {}

N_PREWARM = 14                 # PE warm-up no-op matmuls before the stream


def _build(mode: str) -> bass.Bass:
    f32 = mybir.dt.float32
    f16 = mybir.dt.float16

    nc = bacc.Bacc("TRN2", target_bir_lowering=False, debug=False,
                   num_devices=NCORES)
    tokT = nc.dram_tensor("tokT", [D_MODEL, MS], f16,
                          kind="ExternalInput").ap()
    weff = nc.dram_tensor("weff", [128, KT * D_MODEL], f16,
                          kind="ExternalInput").ap()
    beff = nc.dram_tensor("beff", [128, NDT], f32, kind="ExternalInput").ap()
    outT = nc.dram_tensor("outT", [D_MODEL, MS], f16,
                          kind="ExternalOutput").ap()

    with tile.TileContext(nc) as tc, ExitStack() as ctx:
        weff_pool = ctx.enter_context(tc.tile_pool(name="weff", bufs=1))
        tok_pool = ctx.enter_context(tc.tile_pool(name="tok", bufs=KT))
        bias_pool = ctx.enter_context(tc.tile_pool(name="bias", bufs=1))
        zero_pool = ctx.enter_context(tc.tile_pool(name="zero", bufs=2))
        psum_pool = ctx.enter_context(
            tc.tile_pool(name="psum", bufs=8, space="PSUM"))
        out_pool = ctx.enter_context(tc.tile_pool(name="osb", bufs=16))

        # W_eff halves lead the sync ring; tokens alternate across both rings
        wt = weff_pool.tile([128, KT * D_MODEL], f16, tag="wt")
        half = KT * D_MODEL // 2
        nc.sync.dma_start(wt[:, 0:half], weff[:, 0:half])
        nc.sync.dma_start(wt[:, half:], weff[:, half:])

        toks = []
        for kt in range(KT):
            tk = tok_pool.tile([128, MS], f16, tag="tk")
            eng = nc.scalar if kt % 2 == 0 else nc.sync
            eng.dma_start(tk[:], tokT[kt * 128:(kt + 1) * 128, :])
            toks.append(tk)

        bt = bias_pool.tile([128, NDT], f32, tag="bt")
        nc.scalar.dma_start(bt[:], beff[:, :])

        zmm = zero_pool.tile([128, 128], f16, tag="zmm")
        nc.vector.memset(zmm[:], 0.0)
        zrhs = zero_pool.tile([128, MCHUNK], f16, tag="zrhs")
        nc.vector.memset(zrhs[:], 0.0)

        def evict(ps, dt_i, mi):
            ot = out_pool.tile([128, MCHUNK], f16, name="ot", tag="ot")
            dsl = slice(dt_i * 128, (dt_i + 1) * 128)
            msl = slice(mi * MCHUNK, (mi + 1) * MCHUNK)
            if dt_i % 2 == 0:
                nc.scalar.add(ot[:], ps[:], bt[:, dt_i:dt_i + 1])
                nc.scalar.dma_start(outT[dsl, msl], ot[:])
            else:
                nc.vector.tensor_scalar_add(ot[:], ps[:],
                                            bt[:, dt_i:dt_i + 1])
                nc.sync.dma_start(outT[dsl, msl], ot[:])

        def gen(mi, dts, prewarm):
            psums = [psum_pool.tile([128, MCHUNK], f32, name="ps", tag="ps")
                     for _ in dts]
            if prewarm:
                for _ in range(N_PREWARM):
                    nc.tensor.matmul(psums[0][:], lhsT=zmm[:], rhs=zrhs[:],
                                     start=False, stop=False)
            for kt in range(KT):
                for i, dt_i in enumerate(dts):
                    lhsT = wt[:, kt * D_MODEL + dt_i * 128:
                              kt * D_MODEL + (dt_i + 1) * 128]
                    nc.tensor.matmul(
                        psums[i][:], lhsT=lhsT,
                        rhs=toks[kt][:, mi * MCHUNK:(mi + 1) * MCHUNK],
                        start=(kt == 0), stop=(kt == KT - 1))
                if prewarm and kt < 2:
                    for _ in range(2):
                        nc.tensor.matmul(psums[0][:], lhsT=zmm[:],
                                         rhs=zrhs[:], start=False, stop=False)
            for i, dt_i in enumerate(dts):
                evict(psums[i], dt_i, mi)

        # m-half 0: all 8 d-tiles at once; m-half 1: two 4-bank sub-gens
        gen(0, list(range(NDT)), prewarm=True)
        gen(1, [0, 1, 2, 3], prewarm=False)
        gen(1, [4, 5, 6, 7], prewarm=False)

    nc.compile()
    return nc


def _get_nc(mode: str) -> bass.Bass:
    if mode not in _BUILD_CACHE:
        _BUILD_CACHE[mode] = _build(mode)
    return _BUILD_CACHE[mode]


def _make_in_maps(boundary_tokens, W_b2b, b_b2b):
    wcoef = np.asarray(W_COEF, dtype=np.float32)
    tok = np.asarray(boundary_tokens, dtype=np.float32).reshape(BN, D_MODEL)
    # constant-fold the scan into the weights: W_eff = sum_j w_j W_j
    Weff = (np.asarray(W_b2b, dtype=np.float32).reshape(
        D_MODEL, BULK_DIM, D_MODEL) * wcoef[None, :, None]).sum(
        axis=1, dtype=np.float32)
    beff = (np.asarray(b_b2b, dtype=np.float32).reshape(BULK_DIM, D_MODEL)
            * wcoef[:, None]).sum(axis=0, dtype=np.float32)
    wmap = np.ascontiguousarray(
        Weff.astype(np.float16).reshape(KT, 128, D_MODEL)
        .transpose(1, 0, 2).reshape(128, KT * D_MODEL))
    bmap = np.ascontiguousarray(beff.reshape(NDT, 128).T.astype(np.float32))
    in_maps = []
    for c in range(NCORES):
        in_maps.append({
            "tokT": np.ascontiguousarray(
                tok[c * MS:(c + 1) * MS, :].T.astype(np.float16)),
            "weff": wmap,
            "beff": bmap,
        })
    return in_maps


def _assemble(results):
    out = np.empty((BN, D_MODEL), dtype=np.float32)
    for c in range(NCORES):
        out[c * MS:(c + 1) * MS, :] = results[c]["outT"].T.astype(np.float32)
    return out.reshape(B, N, D_MODEL)


def run(boundary_tokens, W_b2b, b_b2b, mode=None, **spmd_kwargs):
    mode = mode or MODE
    nc = _get_nc(mode)
    in_maps = _make_in_maps(boundary_tokens, W_b2b, b_b2b)
    res = run_bass_kernel_spmd(nc, in_maps, list(range(NCORES)), **spmd_kwargs)
    return _assemble(res.results), res


def kernel(boundary_tokens, W_b2b, b_b2b):
    out, _ = run(boundary_tokens, W_b2b, b_b2b)
    return out
